# revision 15
# baseline (speedup 1.0000x reference)
"""Trainium2 Bass kernel for nn_MoEBlock (8-expert top-2 MoE + shared expert).

Strategy (v2): expert-parallel, binned + chunk-interleaved AllToAll design.
 - Each core owns ONE expert (true expert numbering, no permutation).
 - Home gate: each core computes the fp32 gate only for its 512 "home"
   tokens, derives top-2 masks, per-(expert, block) ranks via matmul prefix
   sums, and bin-local addresses (rank rho or -1). A tiny [8, 512] f32
   AllToAll exchanges the per-expert address columns.
 - Compaction by selection matmuls: XcompT[d, slot] += X_tile^T @ S where
   S[t, col] = (rho(t) == col). No indirect DMA, no DMA transposes; the
   compacted transposed activations land directly in SBUF. Slot layout is
   chunk-interleaved: slot = (rho//32)*256 + block*32 + rho%32, so the
   32-row stripes of each bin psum fan out to the 5 chunk regions.
 - Expert FFN (bf16): W1 resident, FFN1 -> exact Gelu(+b1) -> FFN2 with W2
   streamed per group; outputs land in ycomp rows == interleaved slot ids.
 - Output AllToAll is split into 5 chunks of 256 rows ([8 bins x 32]); each
   fires as soon as its FFN group half completes, overlapping the expert
   FFN. Copy-rate A2A replaces the baseline's ReduceScatter + 650us of
   serial indirect gathers.
 - Destination combine: per home tile compute the two recv-row offsets
   (rho + 224*(rho//32) + expert*32), gather both contributions with
   indirect DMA, scale by the top-2 softmax weights, add the shared-expert
   output and the b2/sb2 term (K=9 matmul on the combine matrix).
 - Shared expert FFN (home tokens) is split around the compaction to keep
   TensorE busy: FFN1 early, FFN2 after the compaction matmuls.
 - DMA queue split: gate/shared streams on sync, X tiles + W1 + W2 streams
   on vector, routing loads + collectives + gathers on gpsimd.
 - Host: concatenates the 8 home slices. Host work is slicing/layout/dtype
   casts of inputs only.
"""

import numpy as np
import ml_dtypes
from contextlib import ExitStack

import concourse.bass as bass
import concourse.tile as tile
from concourse import bacc, mybir
from concourse.bass import IndirectOffsetOnAxis
from concourse.bass_utils import run_bass_kernel_spmd

# Register the axon NTFF profiling hook if the image's antenv lacks it
# (needed only for trace=True; harmless otherwise).
try:
    from antenv.axon_hooks import get_axon_ntff_profile_hook  # noqa: F401
except ImportError:
    try:
        import sys
        import types
        import antenv
        from trn_agent_boot.trn_boot import _ntff_profile_via_ctypes
        _mod = types.ModuleType("antenv.axon_hooks")
        _mod._hook = _ntff_profile_via_ctypes("/opt/axon/libaxon_pjrt.so")
        _mod.get_axon_ntff_profile_hook = lambda: _mod._hook
        _mod.set_axon_ntff_profile_hook = lambda h: setattr(_mod, "_hook", h)
        sys.modules["antenv.axon_hooks"] = _mod
        antenv.axon_hooks = _mod
    except Exception:
        pass

BF16 = ml_dtypes.bfloat16
T, D, INNER, E = 4096, 1024, 4096, 8
N_CORES = 8
HOME = T // N_CORES            # 512
CAPB = 160                     # per-(expert, home-block) bin capacity
CAP = E * CAPB                 # 1280 compacted slots
CHW = 32                       # stripe width (rows per bin per chunk)
NCH = CAPB // CHW              # 5 chunks of 8*32 = 256 rows
KT = D // 128                  # 8 k-tiles of model dim
MT = INNER // 128              # 32 i-tiles of inner dim
GROUPS = [(0, 512), (512, 512), (1024, 256)]   # expert FFN slot groups
CHUNKED_A2A = True             # fire the output A2A per 256-row chunk

_CACHE: dict = {}


def _build_nc(debug: bool = False):
    dt = mybir.dt
    f32, bf, u32 = dt.float32, dt.bfloat16, dt.uint32
    AF = mybir.ActivationFunctionType
    OP = mybir.AluOpType
    AX = mybir.AxisListType

    nc = bacc.Bacc("TRN2", target_bir_lowering=False, debug=False,
                   num_devices=N_CORES)

    def inp(name, shape, dtype):
        return nc.dram_tensor(name, shape, dtype, kind="ExternalInput")

    Xbf_d = inp("Xbf", [T, D], bf)
    XhT32_d = inp("XhT32", [128, KT * HOME], f32)
    XhTbf_d = inp("XhTbf", [128, KT * HOME], bf)
    WgT_d = inp("WgT", [128, KT * E], f32)
    W1_d = inp("W1e", [128, KT * INNER], bf)
    W2_d = inp("W2e", [128, MT * D], bf)
    b1_d = inp("b1e", [128, MT], f32)
    sW1_d = inp("sW1e", [128, MT * KT * 128], bf)
    sW2_d = inp("sW2e", [128, MT * D], bf)
    sb1_d = inp("sb1e", [128, MT], f32)
    b2p_d = inp("b2p9", [9, D], bf)
    utri_d = inp("utri", [128, 128], f32)
    id_d = inp("id128", [128, 128], f32)
    iotaCB_d = inp("iotaCB", [128, CAPB], f32)
    eCB_d = inp("eCB", [128, E], f32)          # col e = e * CHW
    out_d = nc.dram_tensor("out", [HOME, D], f32, kind="ExternalOutput")

    with tile.TileContext(nc) as tc, ExitStack() as ctx:
        const = ctx.enter_context(tc.tile_pool(name="const", bufs=1))
        persist = ctx.enter_context(tc.tile_pool(name="persist", bufs=1))
        stream = ctx.enter_context(tc.tile_pool(name="stream", bufs=4))
        jtp = ctx.enter_context(tc.tile_pool(name="jtp", bufs=4))
        htp = ctx.enter_context(tc.tile_pool(name="htp", bufs=32))
        ypool = ctx.enter_context(tc.tile_pool(name="ypool", bufs=3))
        dram = ctx.enter_context(tc.tile_pool(name="dram", bufs=1, space="DRAM"))
        pph = ctx.enter_context(tc.tile_pool(name="pph", bufs=2, space="PSUM"))
        ppy = ctx.enter_context(tc.tile_pool(name="ppy", bufs=4, space="PSUM"))
        ppt = ctx.enter_context(tc.tile_pool(name="ppt", bufs=2, space="PSUM"))

        # ---- DRAM intermediates ----
        routeS = dram.tile([E, HOME], f32)       # A2A #1 send (addr by expert)
        routeR = dram.tile([E, HOME], f32)       # A2A #1 recv (addr by block)
        ycomp = dram.tile([CAP, D], bf)          # FFN2 out == A2A #2 send
        yrecv = dram.tile([CAP, D], bf)          # A2A #2 recv
        shpark = dram.tile([HOME, D], f32)       # shared-expert FFN output

        # ---- resident constants; gate-critical loads first ----
        WgTsb = const.tile([128, KT * E], f32)
        nc.sync.dma_start(WgTsb[:], WgT_d.ap())
        xh32 = const.tile([128, KT * HOME], f32)
        nc.sync.dma_start(xh32[:], XhT32_d.ap())
        utrisb = const.tile([128, 128], f32)
        nc.sync.dma_start(utrisb[:], utri_d.ap())
        idsb = const.tile([128, 128], f32)
        nc.sync.dma_start(idsb[:], id_d.ap())
        xhbf = const.tile([128, KT * HOME], bf)
        nc.sync.dma_start(xhbf[:], XhTbf_d.ap())
        b1sb = const.tile([128, MT], f32)
        nc.sync.dma_start(b1sb[:], b1_d.ap())
        sb1sb = const.tile([128, MT], f32)
        nc.sync.dma_start(sb1sb[:], sb1_d.ap())
        b2psb = const.tile([9, D], bf)
        nc.sync.dma_start(b2psb[:], b2p_d.ap())
        iotasb = const.tile([128, CAPB], f32)
        nc.sync.dma_start(iotasb[:], iotaCB_d.ap())
        eCBsb = const.tile([128, E], f32)
        nc.sync.dma_start(eCBsb[:], eCB_d.ap())
        W1sb = const.tile([128, KT * INNER], bf)

        # ---- persistent routing state ----
        sjh = persist.tile([128, 4 * E], f32)      # home scores (4 jj tiles)
        t1all = persist.tile([128, 4 * E], f32)    # top-1 masks
        thrall = persist.tile([128, 4 * E], f32)   # top-2 (both) masks
        addrTok = persist.tile([128, 4 * E], f32)  # per-token rank-or-(-1)
        addrTall = persist.tile([8, HOME], f32)    # expert-major addr (send)
        combT = persist.tile([9, HOME], bf)        # combine rows for b2 trick
        addrP = persist.tile([128, 4 * E], f32)    # recv addr, token-major
        routeRsb = persist.tile([8, HOME], f32)
        XcompT = [persist.tile([128, CAP], bf, name=f"XcompT{a}")
                  for a in range(KT)]

        # ---- phase 0: warmup collective (absorb CC first-call latency) ----
        wdin = dram.tile([8, 16], f32)
        wdout = dram.tile([8, 16], f32)
        nc.gpsimd.collective_compute(
            "AllToAll", mybir.AluOpType.bypass,
            replica_groups=[list(range(N_CORES))],
            ins=[wdin[:].opt()], outs=[wdout[:].opt()])

        # ---- phase 1: home gate + routing ----
        _sid = nc.enter_named_scope("p1_gate", False)[0]
        carry = [None] * 5
        carry[0] = jtp.tile([8, 1], f32, tag="carry", bufs=5, name="carry0")
        nc.vector.memset(carry[0][:], 0.0)
        for jj in range(4):
            pg = pph.tile([128, E], f32, tag="ph")
            for a in range(KT):
                nc.tensor.matmul(pg[:],
                                 lhsT=xh32[:, a * HOME + jj * 128:
                                           a * HOME + (jj + 1) * 128],
                                 rhs=WgTsb[:, a * E:(a + 1) * E],
                                 start=(a == 0), stop=(a == KT - 1))
            # softmax
            sj = sjh[:, jj * E:(jj + 1) * E]
            m1n = jtp.tile([128, 1], f32, tag="jt1")
            nc.vector.tensor_reduce(m1n[:], pg[:], axis=AX.X, op=OP.max,
                                    negate=True)
            et = jtp.tile([128, E], f32, tag="jt8")
            nc.scalar.activation(et[:], pg[:], AF.Exp, bias=m1n[:, 0:1])
            ssum = jtp.tile([128, 1], f32, tag="jt1b")
            nc.vector.reduce_sum(ssum[:], et[:], axis=AX.X)
            rcp = jtp.tile([128, 1], f32, tag="jt1c")
            nc.vector.reciprocal(rcp[:], ssum[:])
            nc.vector.tensor_scalar_mul(sj, et[:], rcp[:, 0:1])
            # top-1 / top-2
            t1 = t1all[:, jj * E:(jj + 1) * E]
            thr = thrall[:, jj * E:(jj + 1) * E]
            m1 = jtp.tile([128, 1], f32, tag="jt1d")
            nc.vector.tensor_reduce(m1[:], sj, axis=AX.X, op=OP.max)
            nc.vector.tensor_scalar(t1, sj, m1[:, 0:1], None, op0=OP.is_ge)
            tb = jtp.tile([128, E], f32, tag="jt8b")
            nc.vector.tensor_scalar(tb[:], t1, -1e9, None, op0=OP.mult)
            nc.vector.tensor_tensor(tb[:], tb[:], sj, op=OP.add)
            m2 = jtp.tile([128, 1], f32, tag="jt1e")
            nc.vector.tensor_reduce(m2[:], tb[:], axis=AX.X, op=OP.max)
            nc.vector.tensor_scalar(thr, sj, m2[:, 0:1], None, op0=OP.is_ge)
            # combine rows (for b2/sb2 K=9 matmul): scores*thr, ones
            comb9 = jtp.tile([128, 9], f32, tag="c9")
            nc.vector.tensor_tensor(comb9[:, 0:E], sj, thr, op=OP.mult)
            nc.vector.memset(comb9[:, E:E + 1], 1.0)
            pcT = ppt.tile([9, 128], f32, tag="pt")
            nc.tensor.matmul(pcT[:], lhsT=comb9[:], rhs=idsb[:],
                             start=True, stop=True)
            nc.vector.tensor_copy(combT[0:9, jj * 128:(jj + 1) * 128], pcT[:])
            # ranks: inclusive prefix sum (expert-major) + carry across tiles
            pcs = ppt.tile([8, 128], f32, tag="pt")
            nc.tensor.matmul(pcs[:], lhsT=thr, rhs=utrisb[:],
                             start=True, stop=True)
            inclT = jtp.tile([8, 128], f32, tag="incl")
            nc.vector.tensor_scalar_add(inclT[:], pcs[:], carry[jj][:, 0:1])
            carry[jj + 1] = jtp.tile([8, 1], f32, tag="carry", bufs=5,
                                     name=f"carry{jj + 1}")
            nc.vector.tensor_copy(carry[jj + 1][:], inclT[:, 127:128])
            # maskT (expert-major)
            pmT = ppt.tile([8, 128], f32, tag="pt")
            nc.tensor.matmul(pmT[:], lhsT=thr, rhs=idsb[:],
                             start=True, stop=True)
            # addrT = incl*mask + mask - 1  (rank if routed else -1)
            aT = addrTall[0:8, jj * 128:(jj + 1) * 128]
            nc.vector.tensor_tensor(aT, inclT[:], pmT[:], op=OP.mult)
            nc.vector.tensor_tensor(aT, aT, pmT[:], op=OP.add)
            nc.vector.tensor_scalar_add(aT, aT, -1.0)
            # token-major copy of addr (for the dest-side combine)
            ptk = ppt.tile([128, 8], f32, tag="pt")
            nc.tensor.matmul(ptk[:], lhsT=aT, rhs=idsb[0:8, 0:8],
                             start=True, stop=True)
            nc.vector.tensor_copy(addrTok[:, jj * E:(jj + 1) * E], ptk[:])
        nc.sync.dma_start(routeS[:], addrTall[:])
        nc.leave_named_scope("p1_gate", _sid, False)

        # ---- phase 2: routing AllToAll (+ recv load on gpsimd queue) ----
        _sid = nc.enter_named_scope("p2_route", False)[0]
        nc.gpsimd.collective_compute(
            "AllToAll", mybir.AluOpType.bypass,
            replica_groups=[list(range(N_CORES))],
            ins=[routeS[:].opt()], outs=[routeR[:].opt()])
        nc.gpsimd.dma_start(routeRsb[:], routeR[:])
        nc.leave_named_scope("p2_route", _sid, False)

        # ---- phase 3a: shared expert FFN1 (fills TensorE early) ----
        _sid = nc.enter_named_scope("p3a_sharedF1", False)[0]
        shT = []
        for m in range(MT):
            sw1t = stream.tile([128, KT * 128], bf, tag="sw1", bufs=3,
                               name=f"sw1t{m}")
            nc.sync.dma_start(sw1t[:], sW1_d.ap()[:, m * 1024:(m + 1) * 1024])
            ph = pph.tile([128, HOME], f32, tag="ph")
            for a in range(KT):
                nc.tensor.matmul(ph[:], lhsT=sw1t[:, a * 128:(a + 1) * 128],
                                 rhs=xhbf[:, a * HOME:(a + 1) * HOME],
                                 start=(a == 0), stop=(a == KT - 1))
            ht = htp.tile([128, HOME], bf, tag="ht", name=f"sht{m}")
            nc.scalar.activation(ht[:], ph[:], AF.Gelu, bias=sb1sb[:, m:m + 1])
            shT.append(ht)
        nc.leave_named_scope("p3a_sharedF1", _sid, False)

        # ---- phase 3b: shared expert FFN2 half (emitted per dh) ----
        def shared_ffn2(dh):
            pys = [ppy.tile([128, 512], f32, tag="py", name=f"spys{dh}_{tt}")
                   for tt in range(4)]
            for m in range(MT):
                sw2t = stream.tile([128, 512], bf, tag="sw2", bufs=2,
                                   name=f"sw2t{dh}_{m}")
                nc.sync.dma_start(sw2t[:], sW2_d.ap()[:, m * D + dh * 512:
                                                      m * D + dh * 512 + 512])
                for tt in range(4):
                    nc.tensor.matmul(
                        pys[tt][:], lhsT=shT[m][:, tt * 128:(tt + 1) * 128],
                        rhs=sw2t[:], start=(m == 0), stop=False)
            for tt in range(4):
                nc.tensor.matmul(
                    pys[tt][:], lhsT=combT[0:9, tt * 128:(tt + 1) * 128],
                    rhs=b2psb[0:9, dh * 512:dh * 512 + 512],
                    start=False, stop=True)
                ysh = ypool.tile([128, 512], f32, tag="ysh", bufs=1)
                nc.vector.tensor_copy(ysh[:], pys[tt][:])
                nc.sync.dma_start(
                    shpark[tt * 128:(tt + 1) * 128, dh * 512:dh * 512 + 512],
                    ysh[:])

        _sid = nc.enter_named_scope("p3b_sharedF2a", False)[0]
        shared_ffn2(0)
        nc.leave_named_scope("p3b_sharedF2a", _sid, False)

        # ---- phase 4: compaction by selection matmuls ----
        _sid = nc.enter_named_scope("p4_compact", False)[0]
        for jj in range(4):
            paT = ppt.tile([128, 8], f32, tag="pt")
            nc.tensor.matmul(paT[:],
                             lhsT=routeRsb[0:8, jj * 128:(jj + 1) * 128],
                             rhs=idsb[0:8, 0:8], start=True, stop=True)
            nc.vector.tensor_copy(addrP[:, jj * E:(jj + 1) * E], paT[:])
        for b in range(8):
            xins, Ss = [], []
            for jj in range(4):
                j = b * 4 + jj
                xin = stream.tile([128, D], bf, tag="xin", bufs=4,
                                  name=f"xin{j}")
                nc.scalar.dma_start(xin[:], Xbf_d.ap()[j * 128:(j + 1) * 128, :])
                Sj = stream.tile([128, CAPB], bf, tag="S", bufs=6,
                                 name=f"S{j}")
                nc.vector.tensor_scalar(
                    Sj[:], iotasb[:], addrP[:, jj * E + b:jj * E + b + 1],
                    None, op0=OP.is_equal)
                xins.append(xin)
                Ss.append(Sj)
            for a in range(KT):
                pc = pph.tile([128, CAPB], f32, tag="ph")
                for jj in range(4):
                    nc.tensor.matmul(
                        pc[:], lhsT=xins[jj][:, a * 128:(a + 1) * 128],
                        rhs=Ss[jj][:], start=(jj == 0), stop=(jj == 3))
                # fan the 5 rank-stripes out to the chunk-interleaved columns
                for c in range(NCH):
                    nc.vector.tensor_copy(
                        XcompT[a][:, c * 256 + b * CHW:c * 256 + (b + 1) * CHW],
                        pc[:, c * CHW:(c + 1) * CHW])
        # W1 load on the vector queue: dispatched after the X tiles, done
        # well before the expert FFN needs it.
        nc.scalar.dma_start(W1sb[:], W1_d.ap())
        nc.leave_named_scope("p4_compact", _sid, False)

        # ---- phase 3c: second shared FFN2 half ----
        _sid = nc.enter_named_scope("p3c_sharedF2b", False)[0]
        shared_ffn2(1)
        nc.leave_named_scope("p3c_sharedF2b", _sid, False)

        # ---- phase 5: expert FFN over CAP slots + chunked output A2A ----
        _sid = nc.enter_named_scope("p5_ffn", False)[0]
        for g, (s0, gs) in enumerate(GROUPS):
            hT = []
            for m in range(MT):
                ph = pph.tile([128, gs], f32, tag="ph", name=f"ph{g}_{m}")
                for a in range(KT):
                    nc.tensor.matmul(
                        ph[:], lhsT=W1sb[:, a * INNER + m * 128:
                                         a * INNER + (m + 1) * 128],
                        rhs=XcompT[a][:, s0:s0 + gs],
                        start=(a == 0), stop=(a == KT - 1))
                ht = htp.tile([128, gs], bf, tag="ht", name=f"ht{g}_{m}")
                nc.scalar.activation(ht[:], ph[:], AF.Gelu,
                                     bias=b1sb[:, m:m + 1])
                hT.append(ht)
            ntt = gs // 128
            for dh in range(2):
                pys = [ppy.tile([128, 512], f32, tag="py",
                                name=f"pys{g}_{dh}_{tt}")
                       for tt in range(ntt)]
                for m in range(MT):
                    w2t = stream.tile([128, 512], bf, tag="w2s", bufs=6,
                                      name=f"w2t{g}_{dh}_{m}")
                    nc.scalar.dma_start(
                        w2t[:], W2_d.ap()[:, m * D + dh * 512:
                                          m * D + dh * 512 + 512])
                    for tt in range(ntt):
                        nc.tensor.matmul(
                            pys[tt][:],
                            lhsT=hT[m][:, tt * 128:(tt + 1) * 128],
                            rhs=w2t[:], start=(m == 0), stop=(m == MT - 1))
                for tt in range(ntt):
                    ysb = ypool.tile([128, 512], bf, tag="ysb", bufs=4)
                    nc.scalar.activation(ysb[:], pys[tt][:], AF.Copy)
                    nc.sync.dma_start(
                        ycomp[s0 + tt * 128:s0 + (tt + 1) * 128,
                              dh * 512:dh * 512 + 512], ysb[:])
            # fire the output A2A for the chunks this group completed
            if CHUNKED_A2A:
                for c in range(s0 // 256, (s0 + gs) // 256):
                    nc.gpsimd.collective_compute(
                        "AllToAll", mybir.AluOpType.bypass,
                        replica_groups=[list(range(N_CORES))],
                        ins=[ycomp[c * 256:(c + 1) * 256, :].opt()],
                        outs=[yrecv[c * 256:(c + 1) * 256, :].opt()])
        if not CHUNKED_A2A:
            # unoverlapped variant (all chunk exchanges at the end)
            for c in range(CAP // 256):
                nc.gpsimd.collective_compute(
                    "AllToAll", mybir.AluOpType.bypass,
                    replica_groups=[list(range(N_CORES))],
                    ins=[ycomp[c * 256:(c + 1) * 256, :].opt()],
                    outs=[yrecv[c * 256:(c + 1) * 256, :].opt()])
        nc.leave_named_scope("p5_ffn", _sid, False)

        # ---- phase 7: destination combine ----
        _sid = nc.enter_named_scope("p7_combine", False)[0]
        for jj in range(4):
            sj = sjh[:, jj * E:(jj + 1) * E]
            t1 = t1all[:, jj * E:(jj + 1) * E]
            thr = thrall[:, jj * E:(jj + 1) * E]
            rho = addrTok[:, jj * E:(jj + 1) * E]
            t2 = jtp.tile([128, E], f32, tag="jt8c")
            nc.vector.tensor_tensor(t2[:], thr, t1, op=OP.subtract)
            # interleaved recv row: rho + 224*(rho//32) + e*32
            rm = jtp.tile([128, E], f32, tag="rm")
            nc.vector.tensor_tensor(rm[:], rho, thr, op=OP.mult)
            q32 = jtp.tile([128, E], f32, tag="q32")
            nc.vector.tensor_scalar(q32[:], rm[:], 1.0 / CHW, None,
                                    op0=OP.mult)
            # floor(q32): subtract just under 1/2 then round via the
            # 1.5*2^23 magic constant (f32->u32 copy rounds, so no trunc)
            qt = jtp.tile([128, E], f32, tag="qt")
            nc.vector.tensor_scalar(qt[:], q32[:], -0.484375, 12582912.0,
                                    op0=OP.add, op1=OP.add)
            qf = jtp.tile([128, E], f32, tag="qf")
            nc.vector.tensor_scalar(qf[:], qt[:], -12582912.0, None,
                                    op0=OP.add)
            base = jtp.tile([128, E], f32, tag="jt8d")
            nc.vector.tensor_scalar(base[:], qf[:], float(256 - CHW), None,
                                    op0=OP.mult)
            nc.vector.tensor_tensor(base[:], base[:], rm[:], op=OP.add)
            nc.vector.tensor_tensor(base[:], base[:], eCBsb[:], op=OP.add)
            offw = []
            for tk, tkname in ((t1, "o1"), (t2[:], "o2")):
                q = jtp.tile([128, E], f32, tag="q" + tkname)
                nc.vector.tensor_tensor(q[:], base[:], tk, op=OP.mult)
                offf = jtp.tile([128, 1], f32, tag="f" + tkname)
                nc.vector.reduce_sum(offf[:], q[:], axis=AX.X)
                offu = jtp.tile([128, 1], u32, tag="u" + tkname)
                nc.vector.tensor_copy(offu[:], offf[:])
                w = jtp.tile([128, E], f32, tag="w" + tkname)
                nc.vector.tensor_tensor(w[:], sj, tk, op=OP.mult)
                ws = jtp.tile([128, 1], f32, tag="s" + tkname)
                nc.vector.reduce_sum(ws[:], w[:], axis=AX.X)
                offw.append((offu, ws))
            acc = ypool.tile([128, D], f32, tag="cmb", bufs=2)
            for k, (offu, ws) in enumerate(offw):
                yg = stream.tile([128, D], bf, tag="yg", bufs=2,
                                 name=f"yg{jj}_{k}")
                nc.gpsimd.indirect_dma_start(
                    yg[:], None, yrecv[:],
                    IndirectOffsetOnAxis(ap=offu[:, 0:1], axis=0),
                    bounds_check=CAP - 1, oob_is_err=False)
                if k == 0:
                    nc.scalar.activation(acc[:], yg[:], AF.Copy,
                                         scale=ws[:, 0:1])
                else:
                    ysc = ypool.tile([128, D], f32, tag="cmb2", bufs=1)
                    nc.scalar.activation(ysc[:], yg[:], AF.Copy,
                                         scale=ws[:, 0:1])
                    nc.vector.tensor_tensor(acc[:], acc[:], ysc[:], op=OP.add)
            nc.gpsimd.dma_start(acc[:], shpark[jj * 128:(jj + 1) * 128, :],
                                accum_op=OP.add)
            nc.sync.dma_start(out_d.ap()[jj * 128:(jj + 1) * 128, :], acc[:])
        nc.leave_named_scope("p7_combine", _sid, False)

    nc.compile()
    return nc


def _prep_inputs(hidden_states, Wg, W1, b1, W2, b2, sW1, sb1, sW2, sb2):
    """Host-side sharding/layout: per-core input dicts."""
    X = np.ascontiguousarray(hidden_states.reshape(T, D).astype(np.float32))
    Xbf = X.astype(BF16)
    utri = np.triu(np.ones((128, 128), np.float32))
    id128 = np.eye(128, dtype=np.float32)
    iotaCB = np.tile(np.arange(CAPB, dtype=np.float32), (128, 1))
    eCB = np.tile(np.arange(E, dtype=np.float32) * CHW, (128, 1))
    WgT = np.ascontiguousarray(
        Wg.T.reshape(KT, 128, E).transpose(1, 0, 2)
        .reshape(128, KT * E)).astype(np.float32)
    sW1e = np.ascontiguousarray(
        sW1.reshape(KT, 128, MT, 128).transpose(1, 2, 0, 3)
        .reshape(128, MT * KT * 128)).astype(BF16)
    sW2e = np.ascontiguousarray(
        sW2.reshape(MT, 128, D).transpose(1, 0, 2).reshape(128, MT * D)
    ).astype(BF16)
    sb1e = np.ascontiguousarray(sb1.reshape(MT, 128).T).astype(np.float32)
    b2p9 = np.concatenate([b2, sb2[None, :]], axis=0).astype(np.float32)

    in_maps = []
    for c in range(N_CORES):
        W1e = np.ascontiguousarray(
            W1[c].reshape(KT, 128, INNER).transpose(1, 0, 2)
            .reshape(128, KT * INNER)).astype(BF16)
        W2e = np.ascontiguousarray(
            W2[c].reshape(MT, 128, D).transpose(1, 0, 2).reshape(128, MT * D)
        ).astype(BF16)
        b1e = np.ascontiguousarray(b1[c].reshape(MT, 128).T).astype(np.float32)
        Xh = X[c * HOME:(c + 1) * HOME]
        XhT = np.ascontiguousarray(
            Xh.T.reshape(KT, 128, HOME).transpose(1, 0, 2)
            .reshape(128, KT * HOME))
        in_maps.append({
            "Xbf": Xbf, "XhT32": XhT.astype(np.float32),
            "XhTbf": XhT.astype(BF16), "WgT": WgT,
            "W1e": W1e, "W2e": W2e, "b1e": b1e,
            "sW1e": sW1e, "sW2e": sW2e, "sb1e": sb1e, "b2p9": b2p9.astype(BF16),
            "utri": utri, "id128": id128, "iotaCB": iotaCB, "eCB": eCB,
        })
    return in_maps


def kernel_run(inputs: dict, trace: bool = False, trace_cores=None):
    """Run the SPMD kernel; returns (full_output, BassKernelResults)."""
    if "nc" not in _CACHE:
        _CACHE["nc"] = _build_nc()
    nc = _CACHE["nc"]
    in_maps = _prep_inputs(**{k: np.asarray(v) for k, v in inputs.items()})
    kw = {}
    if trace:
        kw = dict(trace=True,
                  trace_cores=trace_cores if trace_cores is not None else [0])
    res = run_bass_kernel_spmd(nc, in_maps, core_ids=list(range(N_CORES)), **kw)
    out = np.concatenate([res.results[c]["out"] for c in range(N_CORES)],
                         axis=0)
    bsz = inputs["hidden_states"].shape[0]
    return out.reshape(bsz, -1, D).astype(np.float32), res


def kernel(**inputs) -> np.ndarray:
    out, _ = kernel_run(inputs)
    return out
